# revision 33
# baseline (speedup 1.0000x reference)
"""Link-predictor GNN kernel for 8 TRN2 NeuronCores.

Strategy (per sharding hint): shard edges across 8 cores (data parallel),
replicate the bf16-cast node-embedding table + MLP weights on every core.

The gather uses the SWDGE dma_gather ucode (transpose=True), which lands
X^T = emd[idx].T directly in SBUF as [128 d, n_edges] — no PE transposes
and no PSUM->SBUF copies.  dma_gather indices are int16, so nodes are
bucketed into 4 ranges of 25000 rows and edges are classified into 16
(src_bucket, dst_bucket) classes GLOBALLY; each class's edges are dealt
round-robin across the 8 cores, so per-(core, class) counts are
n_k/8 +- 1 (sigma ~24) and a fixed capacity C=4864 is safely above the
max (~4740).  Pad slots use index 0; padded outputs are dropped on the
host, which un-permutes edges back to input order.

Per edge tile: 4 matmuls (K-blocks src/dst x h-blocks 0/1) into two PSUM
tiles, relu on ACT (h0) + DVE tensor_scalar (h1), then — software-
pipelined one tile behind so the PE never head-of-line blocks on the
relu — 2 matmuls for logits and sigmoid on ACT into a per-class output
row, one output DMA per class.
"""

import sys

sys.path.insert(0, "/opt/trn_rl_repo")

import numpy as np
import ml_dtypes

from concourse import bacc, mybir, tile
from concourse.bass_utils import run_bass_kernel_spmd

BF16 = ml_dtypes.bfloat16

N_NODES = 100000
D = 128
H = 256
E_TOTAL = 600000
NCORES = 8

NB = 25000                   # node-bucket width (int16-safe)
NBUCK = 4
NCLS = NBUCK * NBUCK         # 16 (src_bucket, dst_bucket) classes
C = 4864                     # per-(core, class) capacity (multiple of 128)
CT = C // 16                 # idx columns per class in the 16-partition wrap
E_PAD = NCLS * C             # 77824 padded edges per core
TILE_WIDTHS = [512] * 9 + [256]          # per-class tile split (sum = C)
TILE_STARTS = [sum(TILE_WIDTHS[:i]) for i in range(len(TILE_WIDTHS))]
GA = 2560                    # gather-half split for first/last class
GB = C - GA                  # 2304

LAST_RESULTS = None
_NC = None


def _build_program():
    global _NC
    if _NC is not None:
        return _NC
    dt = mybir.dt
    nc = bacc.Bacc(
        "TRN2",
        target_bir_lowering=False,
        debug=False,
        enable_asserts=False,
        num_devices=NCORES,
    )
    emd = nc.dram_tensor("emd", [N_NODES, D], dt.bfloat16, kind="ExternalInput")
    sidx_d = nc.dram_tensor("sidx", [128, NCLS * CT], dt.int16, kind="ExternalInput")
    didx_d = nc.dram_tensor("didx", [128, NCLS * CT], dt.int16, kind="ExternalInput")
    w1_d = nc.dram_tensor("w1", [128, 512], dt.bfloat16, kind="ExternalInput")
    w2_d = nc.dram_tensor("w2", [128, 2], dt.bfloat16, kind="ExternalInput")
    b1_d = nc.dram_tensor("b1", [128, 2], dt.float32, kind="ExternalInput")
    b2_d = nc.dram_tensor("b2", [1, 1], dt.float32, kind="ExternalInput")
    out_d = nc.dram_tensor("out", [NCLS, C], dt.float32, kind="ExternalOutput")

    AF = mybir.ActivationFunctionType
    ALU = mybir.AluOpType

    with tile.TileContext(nc) as tc:
        with (
            tc.tile_pool(name="const", bufs=1) as cpool,
            tc.tile_pool(name="x", bufs=3) as xpool,
            tc.tile_pool(name="h", bufs=4) as hpool,
            tc.tile_pool(name="o", bufs=3) as opool,
            tc.tile_pool(name="ph", bufs=2, space="PSUM") as php,
            tc.tile_pool(name="pl", bufs=2, space="PSUM") as plp,
        ):
            w1_sb = cpool.tile([128, 512], dt.bfloat16)
            nc.sync.dma_start(w1_sb[:, :], w1_d[:, :])
            w2_sb = cpool.tile([128, 2], dt.bfloat16)
            nc.sync.dma_start(w2_sb[:, :], w2_d[:, :])
            b1_sb = cpool.tile([128, 2], dt.float32)
            nc.sync.dma_start(b1_sb[:, :], b1_d[:, :])
            b2_sb = cpool.tile([1, 1], dt.float32)
            nc.sync.dma_start(b2_sb[:, :], b2_d[:, :])
            sidx = cpool.tile([128, NCLS * CT], dt.int16)
            nc.sync.dma_start(sidx[:, :], sidx_d[:, :])
            didx = cpool.tile([128, NCLS * CT], dt.int16)
            nc.sync.dma_start(didx[:, :], didx_d[:, :])

            # one-tile-deep software pipeline for the logits stage:
            # (h0_sb, h1_sb, o_sb, col0, width, store_k or None)
            pending = None

            def flush(p):
                h0_sb, h1_sb, o_sb, c0, w, store_k = p
                l_ps = plp.tile([1, w], dt.float32, tag="l")
                nc.tensor.matmul(
                    l_ps[:, :], lhsT=w2_sb[:, 0:1], rhs=h0_sb[:, :],
                    start=True, stop=False,
                )
                nc.tensor.matmul(
                    l_ps[:, :], lhsT=w2_sb[:, 1:2], rhs=h1_sb[:, :],
                    start=False, stop=True,
                )
                nc.scalar.activation(
                    o_sb[0:1, c0 : c0 + w], l_ps[:, :], AF.Sigmoid,
                    bias=b2_sb[:, 0:1],
                )
                if store_k is not None:
                    nc.sync.dma_start(
                        out_d[store_k : store_k + 1, :], o_sb[:, :]
                    )

            def gather(dst_ap, bucket, idx_tile, col0, n):
                nc.gpsimd.dma_gather(
                    out_ap=dst_ap,
                    in_ap=emd[bucket * NB : (bucket + 1) * NB, :],
                    idxs_ap=idx_tile[:, col0 : col0 + n // 16],
                    num_idxs=n,
                    num_idxs_reg=n,
                    elem_size=D,
                    transpose=True,
                    single_packet=False,
                )

            for k in range(NCLS):
                sb, db = divmod(k, NBUCK)
                xs = xpool.tile([128, 1, C], dt.bfloat16, tag="xs")
                xd = xpool.tile([128, 1, C], dt.bfloat16, tag="xd")
                if k == 0 or k == NCLS - 1:
                    # first class: compute can start after a quarter gather
                    # (shorter startup); last class: the final piece lands
                    # earlier (shorter tail).  Slice-writes into the same
                    # tile keep the pool rotation identical to whole-class
                    # gathers; deps are per overlapping view.
                    splits = [0, GA, C]
                    for a, b in zip(splits, splits[1:]):
                        gather(xs[:, :, a:b], sb, sidx, k * CT + a // 16, b - a)
                        gather(xd[:, :, a:b], db, didx, k * CT + a // 16, b - a)
                else:
                    gather(xs[:, :, :], sb, sidx, k * CT, C)
                    gather(xd[:, :, :], db, didx, k * CT, C)
                o_sb = opool.tile([1, C], dt.float32, tag="o")
                for c0, w in zip(TILE_STARTS, TILE_WIDTHS):
                    rs = xs[:, 0, c0 : c0 + w]
                    rd = xd[:, 0, c0 : c0 + w]
                    h0_ps = php.tile([128, w], dt.float32, tag="h0")
                    h1_ps = php.tile([128, w], dt.float32, tag="h1")
                    nc.tensor.matmul(
                        h0_ps[:, :], lhsT=w1_sb[:, 0:128], rhs=rs,
                        start=True, stop=False,
                    )
                    nc.tensor.matmul(
                        h0_ps[:, :], lhsT=w1_sb[:, 256:384], rhs=rd,
                        start=False, stop=True,
                    )
                    nc.tensor.matmul(
                        h1_ps[:, :], lhsT=w1_sb[:, 128:256], rhs=rs,
                        start=True, stop=False,
                    )
                    nc.tensor.matmul(
                        h1_ps[:, :], lhsT=w1_sb[:, 384:512], rhs=rd,
                        start=False, stop=True,
                    )
                    if pending is not None:
                        flush(pending)
                    h0_sb = hpool.tile([128, w], dt.bfloat16, tag="h0s")
                    h1_sb = hpool.tile([128, w], dt.bfloat16, tag="h1s")
                    nc.scalar.activation(
                        h0_sb[:, :], h0_ps[:, :], AF.Relu, bias=b1_sb[:, 0:1]
                    )
                    nc.vector.tensor_scalar(
                        h1_sb[:, :], h1_ps[:, :],
                        b1_sb[:, 1:2], 0.0, ALU.add, ALU.max,
                    )
                    is_last = c0 + w == C
                    pending = (h0_sb, h1_sb, o_sb, c0, w, k if is_last else None)
            flush(pending)

    nc.compile()
    _NC = nc
    return nc


def _wrap16(flat):
    """[E_PAD] int16 -> [128, NCLS*CT]: class k occupies cols k*CT:(k+1)*CT;
    within a class, gather slot j reads idxs[j % 16, j // 16] of the class
    block (first 16 partitions, replicated to all 8 partition groups)."""
    a = flat.reshape(NCLS, CT, 16).transpose(0, 2, 1)  # [k, p, s]
    b = a.transpose(1, 0, 2).reshape(16, NCLS * CT)
    return np.ascontiguousarray(np.tile(b, (8, 1)))


def _prepare_inputs(emd_all, edge_index, W1, b1, W2, b2):
    emd_bf = np.ascontiguousarray(np.asarray(emd_all, dtype=np.float32)).astype(BF16)
    ei = np.asarray(edge_index).astype(np.int64)
    W1 = np.asarray(W1, dtype=np.float32)
    W2 = np.asarray(W2, dtype=np.float32)
    b1 = np.asarray(b1, dtype=np.float32).reshape(-1)
    b2 = np.asarray(b2, dtype=np.float32).reshape(-1)

    # lhsT blocks: cols 0:256 = W1[:128,:] (src side), 256:512 = W1[128:,:]
    w1_arr = np.concatenate([W1[:D, :], W1[D:, :]], axis=1).astype(BF16)
    w2_arr = np.stack([W2[:128, 0], W2[128:, 0]], axis=1).astype(BF16)
    b1_arr = np.ascontiguousarray(np.stack([b1[:128], b1[128:]], axis=1))
    b2_arr = b2.reshape(1, 1)

    s, d = ei[:, 0], ei[:, 1]
    kcls = (s // NB) * NBUCK + (d // NB)
    counts = np.bincount(kcls, minlength=NCLS)
    order_g = np.argsort(kcls, kind="stable")     # edges grouped by class
    ks = kcls[order_g]
    grp_start = np.zeros(NCLS, np.int64)
    grp_start[1:] = np.cumsum(counts)[:-1]
    pos = np.arange(E_TOTAL) - grp_start[ks]      # position within class
    core = pos % NCORES                           # deal round-robin to cores
    slot = ks * C + pos // NCORES                 # slot on that core
    assert (pos // NCORES).max() < C, f"class overflow: {counts.max()} vs {C * NCORES}"

    in_maps, unshard = [], []
    for c in range(NCORES):
        m = core == c
        eids = order_g[m]                         # global edge ids on core c
        slots = slot[m]
        sflat = np.zeros(E_PAD, np.int16)
        dflat = np.zeros(E_PAD, np.int16)
        sflat[slots] = (s[eids] % NB).astype(np.int16)
        dflat[slots] = (d[eids] % NB).astype(np.int16)
        in_maps.append(
            {
                "emd": emd_bf,
                "sidx": _wrap16(sflat),
                "didx": _wrap16(dflat),
                "w1": w1_arr,
                "w2": w2_arr,
                "b1": b1_arr,
                "b2": b2_arr,
            }
        )
        unshard.append((eids, slots))
    return in_maps, unshard


def kernel(emd_all, edge_index, W1, b1, W2, b2):
    global LAST_RESULTS
    in_maps, unshard = _prepare_inputs(emd_all, edge_index, W1, b1, W2, b2)
    nc = _build_program()
    res = run_bass_kernel_spmd(nc, in_maps, core_ids=list(range(NCORES)))
    LAST_RESULTS = res
    out = np.empty((E_TOTAL,), dtype=np.float32)
    for c in range(NCORES):
        flat = np.asarray(res.results[c]["out"], dtype=np.float32).reshape(-1)
        eids, slots = unshard[c]
        out[eids] = flat[slots]
    return out.reshape(E_TOTAL, 1)


if __name__ == "__main__":
    rng = np.random.default_rng(0)
    emd = rng.standard_normal((N_NODES, D), dtype=np.float32)
    ei = rng.integers(0, N_NODES, size=(E_TOTAL, 2)).astype(np.int32)
    W1 = rng.standard_normal((2 * D, H), dtype=np.float32) / np.sqrt(2 * D)
    W2 = rng.standard_normal((H, 1), dtype=np.float32) / np.sqrt(H)
    out = kernel(emd, ei, W1, np.zeros(H, np.float32), W2, np.zeros(1, np.float32))
    print(out.shape, out[:4, 0])
